# revision 1
# baseline (speedup 1.0000x reference)
"""MoE layer (naive all-experts, top-2 routing) on 8 trn2 NeuronCores.

Strategy: expert-parallel. Core e computes expert e's full MLP over all
tokens, scaled by that expert's (renormalized top-2) gate weight; the
host sums the 8 partial outputs.

Per-core device program (SPMD; per-core data differs):
  - gating logits in fp32 on PE (top-2 selection must match fp32 ref),
    softmax-renormalized top-2 gate extracted for THIS core's expert via
    a host-side permutation that places the core's expert at column 0.
  - expert MLP in bf16 (fp32 PSUM accumulate): hidden^T = relu(W1^T x^T),
    out = (hidden W2 + b2) * gate.  b1 via ACT bias; b2 via a K=1 rank-1
    matmul of ones x b2 accumulated into PSUM.
"""

import numpy as np
import ml_dtypes

B, T_SEQ, D, H, O, E = 2, 2048, 1024, 4096, 1024, 8
T = B * T_SEQ            # 4096 tokens
P = 128                  # partitions
DS = D // P              # 8 d-slices
HS = H // P              # 32 h-slices
TT = 512                 # token tile (free dim of layer-1 matmuls)
NTT = T // TT            # 8 token tiles
TSUB = TT // P           # 4 token subtiles per token tile
OT = 512                 # out free tile
NO = O // OT             # 2
N_CORES = 8


def build_nc(reps=1):
    import concourse.bass as bass
    import concourse.mybir as mybir
    import concourse.tile as tile
    from concourse import bacc

    f32 = mybir.dt.float32
    bf16 = mybir.dt.bfloat16
    AF = mybir.ActivationFunctionType
    ALU = mybir.AluOpType

    nc = bacc.Bacc(None)

    xT = nc.declare_dram_parameter("xT", [D, T], f32, isOutput=False)
    xTb = nc.declare_dram_parameter("xTb", [D, T], bf16, isOutput=False)
    w1 = nc.declare_dram_parameter("w1", [D, H], bf16, isOutput=False)
    w2 = nc.declare_dram_parameter("w2", [H, O], bf16, isOutput=False)
    b1l = nc.declare_dram_parameter("b1l", [P, HS], f32, isOutput=False)
    b2r = nc.declare_dram_parameter("b2r", [1, O], bf16, isOutput=False)
    wgl = nc.declare_dram_parameter("wgl", [P, DS, E], f32, isOutput=False)
    bgr = nc.declare_dram_parameter("bgr", [1, E], f32, isOutput=False)
    onef = nc.declare_dram_parameter("onef", [1, P], f32, isOutput=False)
    oneb = nc.declare_dram_parameter("oneb", [1, P], bf16, isOutput=False)
    out = nc.declare_dram_parameter("out", [T, O], f32, isOutput=True)

    with tile.TileContext(nc) as tc:
        with (
            tc.tile_pool(name="const", bufs=1) as constp,
            tc.tile_pool(name="wpool", bufs=1) as wpool,
            tc.tile_pool(name="xfp", bufs=1) as xfp,
            tc.tile_pool(name="xbp", bufs=2) as xbp,
            tc.tile_pool(name="hidp", bufs=1) as hidp,
            tc.tile_pool(name="gwp", bufs=2) as gwp,
            tc.tile_pool(name="outp", bufs=3) as outp,
            tc.tile_pool(name="pgp", bufs=2, space="PSUM") as pgp,
            tc.tile_pool(name="p1p", bufs=3, space="PSUM") as p1p,
            tc.tile_pool(name="p2p", bufs=3, space="PSUM") as p2p,
        ):
            wgl_t = constp.tile([P, DS, E], f32)
            nc.sync.dma_start(wgl_t[:], wgl[:])
            bgr_t = constp.tile([1, E], f32)
            nc.sync.dma_start(bgr_t[:], bgr[:])
            onef_t = constp.tile([1, P], f32)
            nc.sync.dma_start(onef_t[:], onef[:])
            oneb_t = constp.tile([1, P], bf16)
            nc.sync.dma_start(oneb_t[:], oneb[:])
            b1l_t = constp.tile([P, HS], f32)
            nc.sync.dma_start(b1l_t[:], b1l[:])
            b2r_t = constp.tile([1, O], bf16)
            nc.sync.dma_start(b2r_t[:], b2r[:])

            w1_t = wpool.tile([P, DS, H], bf16)
            nc.sync.dma_start(w1_t[:], w1[:].rearrange("(s p) h -> p s h", p=P))
            w2_t = wpool.tile([P, HS, O], bf16)
            nc.sync.dma_start(w2_t[:], w2[:].rearrange("(s p) o -> p s o", p=P))

            xT_r = xT[:].rearrange("(s p) t -> p s t", p=P)
            xTb_r = xTb[:].rearrange("(s p) t -> p s t", p=P)

            def token_tile(it):
                tok0 = it * TT

                # ---- gating (fp32) for this token tile's TSUB subtiles ----
                xf_t = xfp.tile([P, DS, TT], f32, tag="xf")
                nc.sync.dma_start(xf_t[:], xT_r[:, :, tok0 : tok0 + TT])
                gL = gwp.tile([P, E, TSUB], f32, tag="gL")
                for s in range(TSUB):
                    pg_t = pgp.tile([P, E], f32, tag="pg")
                    for d in range(DS):
                        nc.tensor.matmul(
                            pg_t[:],
                            xf_t[:, d : d + 1, s * P : (s + 1) * P],
                            wgl_t[:, d : d + 1, :],
                            start=(d == 0),
                            stop=False,
                        )
                    nc.tensor.matmul(
                        pg_t[:], onef_t[:], bgr_t[:], start=False, stop=True
                    )
                    nc.scalar.copy(gL[:, :, s : s + 1], pg_t[:])

                # batched top-2 softmax renorm; this core's expert is col 0
                gt = gwp.tile([P, 20, TSUB], f32, tag="gt")
                Lcol = [gL[:, e : e + 1, :] for e in range(E)]
                sl = [gt[:, i : i + 1, :] for i in range(20)]
                vv = nc.vector
                # max tree -> m1 in sl[0]
                vv.tensor_max(sl[1][:], Lcol[0], Lcol[1])
                vv.tensor_max(sl[2][:], Lcol[2], Lcol[3])
                vv.tensor_max(sl[3][:], Lcol[4], Lcol[5])
                vv.tensor_max(sl[4][:], Lcol[6], Lcol[7])
                vv.tensor_max(sl[5][:], sl[1][:], sl[2][:])
                vv.tensor_max(sl[6][:], sl[3][:], sl[4][:])
                vv.tensor_max(sl[0][:], sl[5][:], sl[6][:])
                m1 = sl[0]
                # m2 = max_e( L_e - 1e30*(L_e >= m1) )  in sl[7]
                m2 = sl[7]
                for e in range(E):
                    vv.tensor_tensor(sl[8][:], Lcol[e], m1[:], op=ALU.is_ge)
                    vv.scalar_tensor_tensor(
                        sl[9][:], sl[8][:], -1.0e30, Lcol[e],
                        op0=ALU.mult, op1=ALU.add,
                    )
                    if e == 0:
                        vv.tensor_copy(m2[:], sl[9][:])
                    else:
                        vv.tensor_max(m2[:], m2[:], sl[9][:])
                # gate for col 0:
                keep, dE, d2, eE, e2, den, rec, g0 = (
                    sl[10], sl[11], sl[12], sl[13], sl[14], sl[15], sl[16], sl[17]
                )
                vv.tensor_tensor(keep[:], Lcol[0], m2[:], op=ALU.is_ge)
                vv.tensor_sub(dE[:], Lcol[0], m1[:])
                vv.tensor_sub(d2[:], m2[:], m1[:])
                nc.scalar.activation(eE[:], dE[:], AF.Exp)
                nc.scalar.activation(e2[:], d2[:], AF.Exp)
                vv.tensor_scalar_add(den[:], e2[:], 1.0)
                vv.reciprocal(rec[:], den[:])
                vv.tensor_mul(g0[:], eE[:], rec[:])
                gall = gwp.tile([P, 1, TSUB], f32, tag="gall")
                vv.tensor_mul(gall[:], g0[:], keep[:])

                # ---- layer 1: hidden^T = relu(W1^T x^T + b1) in bf16 ----
                xb_t = xbp.tile([P, DS, TT], bf16, tag="xb")
                nc.sync.dma_start(xb_t[:], xTb_r[:, :, tok0 : tok0 + TT])
                hid_t = hidp.tile([P, HS, TT], bf16, tag="hid")
                for h in range(HS):
                    p1_t = p1p.tile([P, TT], f32, tag="p1")
                    for d in range(DS):
                        nc.tensor.matmul(
                            p1_t[:],
                            w1_t[:, d : d + 1, h * P : (h + 1) * P],
                            xb_t[:, d : d + 1, :],
                            start=(d == 0),
                            stop=(d == DS - 1),
                        )
                    nc.scalar.activation(
                        hid_t[:, h : h + 1, :], p1_t[:], AF.Relu,
                        bias=b1l_t[:, h : h + 1], scale=1.0,
                    )

                # ---- layer 2 + bias + gate scale + store ----
                for t4 in range(TSUB):
                    for o in range(NO):
                        p2_t = p2p.tile([P, OT], f32, tag="p2")
                        for h in range(HS):
                            nc.tensor.matmul(
                                p2_t[:],
                                hid_t[:, h : h + 1, t4 * P : (t4 + 1) * P],
                                w2_t[:, h : h + 1, o * OT : (o + 1) * OT],
                                start=(h == 0),
                                stop=False,
                            )
                        nc.tensor.matmul(
                            p2_t[:], oneb_t[:], b2r_t[:, o * OT : (o + 1) * OT],
                            start=False, stop=True,
                        )
                        out_t = outp.tile([P, OT], f32, tag="outt")
                        nc.scalar.activation(
                            out_t[:], p2_t[:], AF.Copy,
                            scale=gall[:, 0:1, t4 : t4 + 1],
                        )
                        r0 = tok0 + t4 * P
                        nc.sync.dma_start(
                            out[r0 : r0 + P, o * OT : (o + 1) * OT], out_t[:]
                        )

            def main_body():
                for it in range(NTT):
                    token_tile(it)

            if reps == 1:
                main_body()
            else:
                with tc.For_i(0, reps, 1):
                    main_body()

    nc.finalize()
    return nc


class _Runner:
    """Compiled SPMD executor (mirrors bass2jax.run_bass_via_pjrt, but keeps
    the jitted callable so repeat calls don't rebuild/recompile)."""

    def __init__(self, nc):
        import jax
        from jax.experimental.shard_map import shard_map
        from jax.sharding import Mesh, PartitionSpec
        from concourse import bass2jax
        from concourse import mybir

        bass2jax.install_neuronx_cc_hook()
        self.jax = jax
        self.nc = nc

        partition_name = nc.partition_id_tensor.name if nc.partition_id_tensor else None
        in_names, out_names, out_avals, zero_outs = [], [], [], []
        for alloc in nc.m.functions[0].allocations:
            if not isinstance(alloc, mybir.MemoryLocationSet):
                continue
            name = alloc.memorylocations[0].name
            if alloc.kind == "ExternalInput":
                if name != partition_name:
                    in_names.append(name)
            elif alloc.kind == "ExternalOutput":
                out_names.append(name)
                shape = tuple(alloc.tensor_shape)
                dtype = mybir.dt.np(alloc.dtype)
                out_avals.append(jax.core.ShapedArray(shape, dtype))
                zero_outs.append(np.zeros(shape, dtype))
        n_params = len(in_names)
        n_outs = len(out_avals)
        all_in_names = list(in_names) + list(out_names)
        if partition_name is not None:
            all_in_names.append(partition_name)

        self.in_names = in_names
        self.out_names = out_names
        self.out_shapes = [a.shape for a in out_avals]
        self.zero_outs = zero_outs
        self.n_params = n_params

        def _body(*args):
            operands = list(args)
            if partition_name is not None:
                operands.append(bass2jax.partition_id_tensor())
            outs = bass2jax._bass_exec_p.bind(
                *operands,
                out_avals=tuple(out_avals),
                in_names=tuple(all_in_names),
                out_names=tuple(out_names),
                lowering_input_output_aliases=(),
                sim_require_finite=True,
                sim_require_nnan=True,
                nc=nc,
            )
            return tuple(outs)

        devices = jax.devices()[:N_CORES]
        assert len(devices) == N_CORES
        self.mesh = Mesh(np.asarray(devices), ("core",))
        in_specs = (PartitionSpec("core"),) * (n_params + n_outs)
        out_specs = (PartitionSpec("core"),) * n_outs
        self.sharded = jax.jit(
            shard_map(
                _body, mesh=self.mesh, in_specs=in_specs, out_specs=out_specs,
                check_rep=False,
            ),
            keep_unused=True,
        )

    def prepare(self, in_maps):
        """Concatenate per-core inputs along axis 0 and device_put."""
        concat_in = [
            np.concatenate([np.asarray(m[name]) for m in in_maps], axis=0)
            for name in self.in_names
        ]
        concat_zeros = [
            np.zeros((N_CORES * z.shape[0], *z.shape[1:]), z.dtype)
            for z in self.zero_outs
        ]
        return concat_in + concat_zeros

    def run_prepared(self, args):
        out_arrs = self.sharded(*args)
        self.jax.block_until_ready(out_arrs)
        return out_arrs

    def run(self, in_maps):
        out_arrs = self.run_prepared(self.prepare(in_maps))
        res = []
        for c in range(N_CORES):
            res.append({
                name: np.asarray(out_arrs[i]).reshape(
                    N_CORES, *self.out_shapes[i]
                )[c]
                for i, name in enumerate(self.out_names)
            })
        return res


_RUNNER = None


def get_runner(reps=1):
    global _RUNNER
    if reps != 1:
        return _Runner(build_nc(reps))
    if _RUNNER is None:
        _RUNNER = _Runner(build_nc())
    return _RUNNER


def make_in_maps(x, Wg, bg, W1, b1, W2, b2):
    """Host-side shard/layout prep. x:[B,T,D] f32, Wg:[D,E], bg:[E],
    W1:[E,D,H], b1:[E,H], W2:[E,H,O], b2:[E,O]."""
    bf = ml_dtypes.bfloat16
    x = np.asarray(x, dtype=np.float32).reshape(T, D)
    Wg = np.asarray(Wg, dtype=np.float32)
    bg = np.asarray(bg, dtype=np.float32)
    W1 = np.asarray(W1)
    b1 = np.asarray(b1, dtype=np.float32)
    W2 = np.asarray(W2)
    b2 = np.asarray(b2)

    xT = np.ascontiguousarray(x.T)                      # [D, T] f32
    xTb = xT.astype(bf)                                 # [D, T] bf16
    onef = np.ones((1, P), np.float32)
    oneb = np.ones((1, P), bf)

    in_maps = []
    for e in range(E):
        perm = [e] + [i for i in range(E) if i != e]
        wg_p = Wg[:, perm]                              # [D, E], col 0 = expert e
        wgl = np.ascontiguousarray(
            wg_p.reshape(DS, P, E).transpose(1, 0, 2)
        )                                               # [P, DS, E]
        in_maps.append({
            "xT": xT,
            "xTb": xTb,
            "w1": np.asarray(W1[e], np.float32).astype(bf),
            "w2": np.asarray(W2[e], np.float32).astype(bf),
            "b1l": np.ascontiguousarray(b1[e].reshape(HS, P).T),
            "b2r": np.asarray(b2[e], np.float32).astype(bf).reshape(1, O),
            "wgl": wgl,
            "bgr": bg[perm].reshape(1, E),
            "onef": onef,
            "oneb": oneb,
        })
    return in_maps


def kernel(x, Wg, bg, W1, b1, W2, b2, num_experts_per_tok):
    assert int(num_experts_per_tok) == 2
    runner = get_runner()
    in_maps = make_in_maps(x, Wg, bg, W1, b1, W2, b2)
    results = runner.run(in_maps)
    acc = results[0]["out"].astype(np.float32)
    for c in range(1, N_CORES):
        acc = acc + results[c]["out"]
    return acc.reshape(B, T_SEQ, O)

